# revision 39
# baseline (speedup 1.0000x reference)
"""Trainium2 Bass kernel for nn_NodeEmbedding (GNN message passing).

Strategy (edge sharding, no collectives), v2.1:
  - Host: sort edges by destination row; split the 50k nodes across 8 cores
    (6250 each), group edges into 128-node windows padded to a uniform
    per-window chunk count so the SPMD program is static.
  - Neighbor-embedding rows are looked up ON DEVICE: the per-edge neighbor
    type vector zc [1,EP] is DMA-broadcast across partitions, compared
    against a partition-index constant to build the one-hot
    S_T[t,e] = (zc[e]==t), and nr[e,h] = S_T.T @ nemb on the PE.
  - Cutoff C (and the projection bias when nonzero) is folded into the
    transposed edge-feature matrix eaT (bf16): W = eaT.T @ P65 on the PE.
  - msg = W * nr (DVE); segment_sum via one-hot(node-in-window) matmul:
    aggT[h, n] += msg[e,h].T @ oh[e,n].
  - combine: out[n,o] = aggT.T @ W2.T + (atom_emb@W1.T + b)[z[n]]; the node
    table T1 is applied on device by a second accumulation matmul
    S_z.T @ T1 into the same output PSUM (S_z one-hot built like S_T).
  - DMA count is kept low (~25/core): edge data streams in 13-supertile
    groups, all small constants ride one merged tensor, outputs are written
    13 windows per DMA.
"""

import os
import sys

import numpy as np

for p in ("/opt/trn_rl_repo",):
    if p not in sys.path and os.path.isdir(p):
        sys.path.insert(0, p)

import ml_dtypes

N_NODES = 50000
N_EDGES = 800000
H = 128
RBF = 64
CUTOFF = 5.0
MAX_Z = 100
NT = MAX_Z + 1  # 101 types
NCORES = 8
NPC = N_NODES // NCORES  # 6250 nodes per core
WIN = 128
NW = 52  # windows per core (52*128 = 6656 >= 6250; NW*CW % SC == 0 for even CW)
NLP = NW * WIN  # 6656 padded nodes per core
SC = 8  # chunks (of 128 edges) per supertile
GRP = 13  # supertiles per edge-DMA group (NST = 6.5*CW is divisible by 13)

TRACE = False
LAST_PERF = {}


def _prep(z, edge_index, edge_dist, edge_attr, nrows):
    """Sort/shard/pad edges; returns per-core arrays + layout constants.

    nrows is 64 when proj_b is all-zero (the bias row b*C contributes
    nothing and is dropped), else 65.
    """
    f32 = np.float32
    bf16 = ml_dtypes.bfloat16
    row = np.asarray(edge_index[0], dtype=np.int64)
    col = np.asarray(edge_index[1], dtype=np.int64)
    d = np.asarray(edge_dist, dtype=f32)
    C = (0.5 * (np.cos(np.pi * d / CUTOFF) + 1.0)).astype(f32) * (d < CUTOFF)
    ea = np.asarray(edge_attr, dtype=f32)
    eaC = np.empty((N_EDGES, nrows), dtype=f32)
    eaC[:, :RBF] = ea * C[:, None]
    if nrows > RBF:
        eaC[:, RBF] = C
    zc = np.asarray(z, dtype=np.int64)[col].astype(np.int32)

    core_of = row // NPC
    local = row - core_of * NPC

    # Balance windows: nodes are assigned to (window, slot) per core by
    # degree-sorted round-robin, flattening the max edges-per-window (and
    # hence CW) to just above the mean instead of the node-order max.
    winof = np.empty((NCORES, NPC), dtype=np.int64)
    slotof = np.empty((NCORES, NPC), dtype=np.int64)
    for i in range(NCORES):
        deg = np.bincount(local[core_of == i], minlength=NPC)
        nodeorder = np.argsort(-deg, kind="stable")
        winof[i][nodeorder] = np.arange(NPC) % NW
        slotof[i][nodeorder] = np.arange(NPC) // NW

    cw_key = core_of * NW + winof[core_of, local]
    order = np.argsort(cw_key, kind="stable")
    eaC_s = eaC[order]
    zc_s = zc[order]
    key_s = cw_key[order]
    core_s = key_s // NW
    w_s = key_s % NW
    rel = slotof[core_of, local][order].astype(f32)

    counts = np.bincount(cw_key, minlength=NCORES * NW)
    ewmax = int(counts.max())
    CW = (ewmax + 127) // 128
    CW += CW % 2  # NW*CW % SC == 0 needs CW even
    EW = CW * 128
    CH = NW * CW
    EP = CH * 128

    starts = np.zeros(NCORES * NW + 1, dtype=np.int64)
    np.cumsum(counts, out=starts[1:])
    off_in_win = np.arange(len(order), dtype=np.int64) - starts[key_s]
    dest = w_s * EW + off_in_win  # within-core flat slot

    eaT = np.zeros((NCORES, nrows, EP), dtype=bf16)
    zcr = np.zeros((NCORES, EP), dtype=bf16)
    rloc = np.zeros((NCORES, EP), dtype=bf16)
    for i in range(NCORES):
        m = core_s == i
        eaT[i][:, dest[m]] = eaC_s[m].T.astype(bf16)
        zcr[i][dest[m]] = zc_s[m].astype(bf16)
        rloc[i][dest[m]] = rel[m].astype(bf16)
    # rloc: [EP] -> [128, CH] with flat = c*128 + p
    rloc = np.ascontiguousarray(rloc.reshape(NCORES, CH, 128).transpose(0, 2, 1))
    # node -> padded output row (window*128 + slot), per core
    nidx = winof * 128 + slotof
    return eaT, zcr, rloc, nidx, CW, CH, EP


def _split_waits(nc):
    """Hoist excess sem-waits onto same-engine NoOps.

    The axon walrus toolchain accepts very few sync-wait slots per
    instruction; a NoOp issued just before on the same engine satisfies the
    wait in program order instead.
    """
    import concourse.mybir as mybir

    k = 0
    for fn in nc.m.functions:
        for bb in fn.blocks:
            il = bb.instructions
            i = 0
            while i < len(il):
                inst = il[i]
                si = inst.sync_info
                if si is not None and si.on_wait and len(si.on_wait) > 1:
                    waits = list(si.on_wait)
                    keep, excess = waits[:1], waits[1:]
                    for w in excess:
                        nop = mybir.InstNoOp(name=f"wsplit-{k}")
                        k += 1
                        nop.engine = inst.engine
                        nop.sync_info = mybir.SyncInfo(
                            on_wait=[w], on_update=[]
                        )
                        il.insert(i, nop)
                        i += 1
                    inst.sync_info = mybir.SyncInfo(
                        on_wait=keep, on_update=list(si.on_update or [])
                    )
                i += 1


def _build_program(CW, CH, EP, nrows):
    import concourse.bass as bass
    import concourse.mybir as mybir
    import concourse.tile as tile

    f32 = mybir.dt.float32
    bf16 = mybir.dt.bfloat16
    NST = CH // SC
    NG = NST // GRP
    SE = SC * 128  # edges per supertile
    GE = GRP * SE  # edges per DMA group
    # merged constant layout (free-dim offsets in a [128, CC] bf16 tensor)
    o_rloc = 0
    o_iota = o_rloc + CH
    o_tit = o_iota + SE
    o_nemb = o_tit + 1
    o_w2 = o_nemb + H
    o_t1 = o_w2 + H
    CC = o_t1 + H

    nc = bass.Bass()
    ea_d = nc.dram_tensor("eaT", [nrows, EP], bf16, kind="ExternalInput")
    zcr_d = nc.dram_tensor("zcr", [1, EP], bf16, kind="ExternalInput")
    zn_d = nc.dram_tensor("znr", [1, NLP], bf16, kind="ExternalInput")
    cc_d = nc.dram_tensor("cc", [128, CC], bf16, kind="ExternalInput")
    p65_d = nc.dram_tensor("p65", [nrows, H], bf16, kind="ExternalInput")
    out_d = nc.dram_tensor("outT", [NLP, H], bf16, kind="ExternalOutput")
    # device-local staging for the rows that broadcast DMAs re-read 128x
    # (keeps repeated reads off the possibly host-resident input buffers)
    zs_d = nc.dram_tensor("zs", [1, EP], bf16, kind="Internal")
    zns_d = nc.dram_tensor("zns", [1, NLP], bf16, kind="Internal")

    with tile.TileContext(nc) as tc:
        with (
            tc.tile_pool(name="const", bufs=1) as cp,
            tc.tile_pool(name="ea", bufs=2) as eap,
            tc.tile_pool(name="zb", bufs=2) as zbp,
            tc.tile_pool(name="st", bufs=2) as stp,
            tc.tile_pool(name="wb", bufs=2) as wbp,
            tc.tile_pool(name="msg", bufs=2) as msp,
            tc.tile_pool(name="oh", bufs=2) as ohp,
            tc.tile_pool(name="wind", bufs=2) as wnp,
            tc.tile_pool(name="wps", bufs=1, space="PSUM") as wps,
            tc.tile_pool(name="nrps", bufs=2, space="PSUM") as nrps,
            tc.tile_pool(name="aggp", bufs=1, space="PSUM") as aggp,
            tc.tile_pool(name="outp", bufs=1, space="PSUM") as outp,
        ):
            cc_t = cp.tile([128, CC], bf16, tag="cc")
            nc.sync.dma_start(cc_t[:], cc_d[:])
            p65_t = cp.tile([nrows, H], bf16, tag="p65")
            nc.sync.dma_start(p65_t[:], p65_d[:])
            rloc_t = cc_t[:, o_rloc : o_rloc + CH]
            iota_t = cc_t[:, o_iota : o_iota + SE].rearrange(
                "p (s j) -> p s j", s=SC
            )
            titer_t = cc_t[:, o_tit : o_tit + 1]
            nemb_t = cc_t[:, o_nemb : o_nemb + H]
            w2_t = cc_t[:, o_w2 : o_w2 + H]
            t1_t = cc_t[:, o_t1 : o_t1 + H]
            nc.sync.dma_start(zs_d[:], zcr_d[:])
            nc.sync.dma_start(zns_d[:], zn_d[:])
            zbn_t = cp.tile([128, NLP], bf16, tag="zbn")
            nc.sync.dma_start(
                zbn_t[:], zns_d[:].broadcast_to((128, NLP))
            )

            tc.strict_bb_all_engine_barrier()

            agg = [None]
            obw = [None]
            for g in range(NG):
                ge0 = g * GE
                ea_g = eap.tile([nrows, GE], bf16, tag="ea")
                nc.sync.dma_start(ea_g[:], ea_d[:, ge0 : ge0 + GE])
                zbc_g = zbp.tile([128, GE], bf16, tag="zb")
                nc.sync.dma_start(
                    zbc_g[:],
                    zs_d[:, ge0 : ge0 + GE].broadcast_to((128, GE)),
                )
                for si in range(GRP):
                    st = g * GRP + si
                    s0 = si * SE
                    st_t = stp.tile([128, SE], bf16, tag="st")
                    nc.vector.tensor_tensor(
                        st_t[:],
                        zbc_g[:, s0 : s0 + SE],
                        titer_t.broadcast_to((128, SE)),
                        op=mybir.AluOpType.is_equal,
                    )
                    wt = wps.tile([128, SE], f32, tag="w")
                    for j in range(SC):
                        nc.tensor.matmul(
                            wt[:, j * 128 : (j + 1) * 128],
                            ea_g[:, s0 + j * 128 : s0 + (j + 1) * 128],
                            p65_t[:],
                            start=True,
                            stop=True,
                        )
                    nrt = nrps.tile([128, SE], f32, tag="nr")
                    for j in range(SC):
                        nc.tensor.matmul(
                            nrt[:, j * 128 : (j + 1) * 128],
                            st_t[:, j * 128 : (j + 1) * 128],
                            nemb_t,
                            start=True,
                            stop=True,
                        )
                    # ACT evicts W PSUM as bf16; DVE multiplies with nr PSUM
                    wb = wbp.tile([128, SE], bf16, tag="wb")
                    nc.scalar.copy(wb[:], wt[:])
                    ms = msp.tile([128, SE], bf16, tag="ms")
                    nc.vector.tensor_tensor(
                        ms[:], wb[:], nrt[:], op=mybir.AluOpType.mult
                    )
                    oh = ohp.tile([128, SC, 128], bf16, tag="oh")
                    rl = rloc_t[:, st * SC : (st + 1) * SC].unsqueeze(-1)
                    nc.vector.tensor_tensor(
                        oh[:],
                        iota_t,
                        rl.broadcast_to((128, SC, 128)),
                        op=mybir.AluOpType.is_equal,
                    )
                    for j in range(SC):
                        c = st * SC + j
                        w = c // CW
                        if c % CW == 0:
                            agg[0] = aggp.tile(
                                [128, 128], f32, tag="agg", name=f"agg{w}"
                            )
                        nc.tensor.matmul(
                            agg[0][:],
                            ms[:, j * 128 : (j + 1) * 128],
                            oh[:, j, :],
                            start=(c % CW == 0),
                            stop=(c % CW == CW - 1),
                        )
                        if c % CW == CW - 1:
                            ag = wnp.tile([128, 128], bf16, tag="ag")
                            nc.vector.tensor_copy(ag[:], agg[0][:])
                            sz = wnp.tile([128, 128], bf16, tag="sz")
                            nc.vector.tensor_tensor(
                                sz[:],
                                zbn_t[:, w * 128 : (w + 1) * 128],
                                titer_t.broadcast_to((128, 128)),
                                op=mybir.AluOpType.is_equal,
                            )
                            ot = outp.tile([128, 128], f32, tag="ot")
                            nc.tensor.matmul(
                                ot[:], ag[:], w2_t, start=True, stop=False
                            )
                            nc.tensor.matmul(
                                ot[:], sz[:], t1_t, start=False, stop=True
                            )
                            if w % 13 == 0:
                                obw[0] = wnp.tile(
                                    [128, 13, 128], bf16, tag="ob",
                                    name=f"ob{w // 13}",
                                )
                            nc.vector.tensor_copy(
                                obw[0][:, w % 13, :], ot[:]
                            )
                            if w % 13 == 12:
                                nc.sync.dma_start(
                                    out_d[
                                        (w - 12) * 128 : (w + 1) * 128, :
                                    ].rearrange("(c p) h -> p c h", p=128),
                                    obw[0][:],
                                )
    _split_waits(nc)
    return nc


def kernel(z, edge_index, edge_dist, edge_attr, atom_emb, neighbor_emb,
           proj_W, proj_b, comb_W, comb_b):
    from concourse.bass_utils import run_bass_kernel_spmd

    f32 = np.float32
    bf16 = ml_dtypes.bfloat16
    z = np.asarray(z)
    edge_index = np.asarray(edge_index)
    edge_dist = np.asarray(edge_dist)
    edge_attr = np.asarray(edge_attr)
    atom_emb = np.asarray(atom_emb, dtype=f32)
    neighbor_emb = np.asarray(neighbor_emb, dtype=f32)
    proj_W = np.asarray(proj_W, dtype=f32)
    proj_b = np.asarray(proj_b, dtype=f32)
    comb_W = np.asarray(comb_W, dtype=f32)
    comb_b = np.asarray(comb_b, dtype=f32)

    nrows = RBF if not proj_b.any() else RBF + 1
    eaT, zcr, rloc, nidx, CW, CH, EP = _prep(
        z, edge_index, edge_dist, edge_attr, nrows
    )
    nc = _build_program(CW, CH, EP, nrows)

    T1 = (atom_emb @ comb_W[:, :H].T + comb_b).astype(f32)  # [101, 128]
    w2t = np.ascontiguousarray(comb_W[:, H:].T).astype(bf16)  # [h_in, out]
    p65 = np.concatenate([proj_W.T, proj_b[None, :]], axis=0)[:nrows].astype(
        bf16
    )
    nembp = np.zeros((128, H), dtype=bf16)
    nembp[:NT] = neighbor_emb.astype(bf16)
    titer = np.arange(128, dtype=f32).astype(bf16)[:, None]
    iota = np.tile(np.arange(128, dtype=f32)[None, :], (128, SC)).astype(bf16)

    zarr = np.asarray(z, dtype=np.int64)
    T1p = np.zeros((128, H), dtype=bf16)
    T1p[:NT] = T1.astype(bf16)

    in_maps = []
    for i in range(NCORES):
        # zn[window*128 + slot] = z of the node mapped there (pads -> 0)
        zn = np.zeros(NLP, dtype=np.int64)
        zn[nidx[i]] = zarr[i * NPC : (i + 1) * NPC]
        cc = np.concatenate(
            [rloc[i], iota, titer, nembp, w2t, T1p], axis=1
        )
        in_maps.append(
            {
                "eaT": np.ascontiguousarray(eaT[i]),
                "zcr": zcr[i][None, :],
                "znr": zn.astype(f32).astype(bf16)[None, :],
                "cc": np.ascontiguousarray(cc),
                "p65": p65,
            }
        )

    try:
        res = run_bass_kernel_spmd(
            nc, in_maps, core_ids=list(range(NCORES)), trace=TRACE
        )
    except Exception:
        # one retry: the axon worker occasionally reports a stale
        # "unrecoverable" state from a previous process's crash
        res = run_bass_kernel_spmd(
            nc, in_maps, core_ids=list(range(NCORES)), trace=TRACE
        )
    LAST_PERF.clear()
    LAST_PERF.update(
        exec_time_ns=res.exec_time_ns,
        mean_exec_time_ns=res.mean_exec_time_ns,
        trace=getattr(res, "instructions_and_trace", None),
        layout=(CW, CH, EP, nrows),
    )

    out = np.empty((N_NODES, H), dtype=f32)
    for i in range(NCORES):
        out[i * NPC : (i + 1) * NPC] = (
            res.results[i]["outT"][nidx[i]].astype(f32)
        )
    return out
